# revision 14
# baseline (speedup 1.0000x reference)
"""Trainium2 Bass kernel for nn_PhenoConnect7 (dense_mlp, memory-bound).

Math: the reference computes, per batch row b and channel c (C=32, H=16384):
    x1 = A_c*gh1 + B_c*h1 + K_c        (A = s, B = s*exp(gen_bias),
    x2 = A_c*gh2 + B_c*h2 + K_c         K = G_LEN*s*exp(hpo_bias)*(1+exp(gen_bias)))
    x3[b,c] = sum_h x1*x2*p2[c,h]       (p2 = hpo_par^2)
    out = sqrt(x3)*out_scale + out_bias ; tiny MLP head -> sigmoid -> (B,1)

Expanding x1*x2 turns x3 into 8 matmuls against p2 plus a per-channel constant:
    x3 = A^2*M11 + A*B*(M12+M21) + B^2*M22
       + A*K*(Vg1+Vg2) + B*K*(Vh1+Vh2) + K^2*P
with M.. = sum_h (prod of two inputs)*p2, V.. = sum_h input*p2, P = sum_h p2.

Kernel strategy (pure batch-parallel across 8 cores, 32 rows each):
  - DMA inputs f32->bf16 (SWDGE cast) into a 4-tensor partition stack
    (128 part = 4 tensors x 32 batch rows, free = H chunk).
  - PE transposes 128x128 stack tiles -> h-partitioned tiles in PSUM.
  - ACT copies raw transposed tiles PSUM->SBUF; DVE forms the 4 pairwise
    products directly from PSUM into SBUF (interleaved [raw|prod] layout).
  - p2 = hpo_par^2 computed on DVE (bf16 out + exact f32 row-sum partials for
    the dominant K^2*P term), PE-transposed to p2T (h-part, c-free).
  - Mains: per 128-h tile one matmul, stationary p2T (128,32), moving the
    256-col [rawT|prodT] stack; PSUM (32c, 256) accumulates all 9 sums.
  - Finalize on (32,32): combine with per-channel coeffs, sqrt, MLP head,
    sigmoid, DMA out (32,1) per core.
"""

import os
import numpy as np

B_FULL = 256
H = 16384
C = 32
N_CORES = 8
B_CORE = B_FULL // N_CORES  # 32
G_LEN = 20000.0

H_CHUNK = 4096
N_CHUNKS = H // H_CHUNK          # 4
TILES_PER_CHUNK = H_CHUNK // 128  # 32 h-tiles of 128
QUADS_PER_CHUNK = TILES_PER_CHUNK // 8  # 4 psum quad-tiles (8 transposes each)

_cache = {}


def _build():
    import concourse.bass as bass
    import concourse.mybir as mybir
    from concourse import bacc
    from concourse.tile import TileContext
    from concourse.masks import make_identity

    f32 = mybir.dt.float32
    bf16 = mybir.dt.bfloat16
    Alu = mybir.AluOpType
    Act = mybir.ActivationFunctionType

    nc = bacc.Bacc()

    gh1 = nc.dram_tensor("gh1", [B_CORE, H], f32, kind="ExternalInput")
    h1 = nc.dram_tensor("h1", [B_CORE, H], f32, kind="ExternalInput")
    gh2 = nc.dram_tensor("gh2", [B_CORE, H], f32, kind="ExternalInput")
    h2 = nc.dram_tensor("h2", [B_CORE, H], f32, kind="ExternalInput")
    hpo = nc.dram_tensor("hpo", [C, H], f32, kind="ExternalInput")
    w1 = nc.dram_tensor("w1", [2 * C, C], f32, kind="ExternalInput")
    # vec rows: 0 out_scale, 1 out_bias, 2 genes_hpo_scale, 3 hpo_bias,
    # 4 gen_bias, 5-6 b1, 7 [b2,...], 8-9 W2
    vec = nc.dram_tensor("vec", [10, C], f32, kind="ExternalInput")
    out = nc.dram_tensor("out", [B_CORE, 1], f32, kind="ExternalOutput")

    raws = [gh1, h1, gh2, h2]

    with TileContext(nc) as tc:
        with (
            tc.tile_pool(name="const", bufs=1) as constp,
            tc.tile_pool(name="stack", bufs=3) as stackp,
            tc.tile_pool(name="p2s", bufs=3) as p2sp,
            tc.tile_pool(name="mv", bufs=2) as mvp,
            tc.tile_pool(name="small", bufs=1) as smallp,
            tc.tile_pool(name="pst", bufs=3, space="PSUM") as pstp,
            tc.tile_pool(name="psp2", bufs=2, space="PSUM") as psp2p,
            tc.tile_pool(name="psmain", bufs=1, space="PSUM") as psmainp,
            tc.tile_pool(name="psmlp", bufs=1, space="PSUM") as psmlpp,
        ):
            ident_bf = constp.tile([128, 128], bf16)
            make_identity(nc, ident_bf)

            # ---- small parameter loads ----
            vec_sb = smallp.tile([C, 10], f32)  # vec transposed: col r = vec row r
            nc.sync.dma_start(
                out=vec_sb, in_=bass.AP(vec, 0, [[1, C], [C, 10]])
            )
            b1_sb = smallp.tile([2 * C, 1], f32)
            nc.sync.dma_start(out=b1_sb, in_=bass.AP(vec, 5 * C, [[1, 2 * C], [1, 1]]))
            w2_sb = smallp.tile([2 * C, 1], f32)
            nc.sync.dma_start(out=w2_sb, in_=bass.AP(vec, 8 * C, [[1, 2 * C], [1, 1]]))
            b2_row = smallp.tile([1, 1], f32)
            nc.sync.dma_start(out=b2_row, in_=bass.AP(vec, 7 * C, [[1, 1], [1, 1]]))
            b2_sb = smallp.tile([C, 1], f32)
            nc.gpsimd.partition_broadcast(b2_sb, b2_row)

            # W1T (32,64) loaded directly with a strided (transposing) DMA --
            # tiny tensor, avoids an f32 PE matmul (1-wait codegen limit)
            w1t_sb = smallp.tile([C, 2 * C], f32)
            nc.sync.dma_start(out=w1t_sb, in_=bass.AP(w1, 0, [[1, C], [C, 2 * C]]))

            # ---- per-channel coefficients (32,1 each) ----
            s_ap = vec_sb[:, 2:3]
            eg = smallp.tile([C, 1], f32)
            nc.scalar.activation(eg, vec_sb[:, 4:5], Act.Exp)
            eh = smallp.tile([C, 1], f32)
            nc.scalar.activation(eh, vec_sb[:, 3:4], Act.Exp)
            coefB = smallp.tile([C, 1], f32)
            nc.vector.scalar_tensor_tensor(
                coefB, in0=s_ap, scalar=1.0, in1=eg, op0=Alu.bypass, op1=Alu.mult
            )
            # K = G_LEN * s * eh * (1+eg)
            t1 = smallp.tile([C, 1], f32)
            nc.vector.scalar_tensor_tensor(
                t1, in0=eg, scalar=1.0, in1=eh, op0=Alu.add, op1=Alu.mult
            )
            coefK = smallp.tile([C, 1], f32)
            nc.vector.scalar_tensor_tensor(
                coefK, in0=t1, scalar=G_LEN, in1=s_ap, op0=Alu.mult, op1=Alu.mult
            )

            def mul2(name, a, b):
                t = smallp.tile([C, 1], f32, name=name)
                nc.vector.scalar_tensor_tensor(
                    t, in0=a, scalar=1.0, in1=b, op0=Alu.bypass, op1=Alu.mult
                )
                return t

            cA2 = mul2("cA2", s_ap, s_ap)
            cAB = mul2("cAB", s_ap, coefB)
            cB2 = mul2("cB2", coefB, coefB)
            cAK = mul2("cAK", s_ap, coefK)
            cBK = mul2("cBK", coefB, coefK)
            cK2 = mul2("cK2", coefK, coefK)

            # ---- main loop over H chunks ----
            psum_main = psmainp.tile([C, 256], f32)
            ppart_tiles = []

            for ch in range(N_CHUNKS):
                h0 = ch * H_CHUNK

                # input stack: partition 32q+j = tensor q, batch row j (bf16)
                in_stack = stackp.tile([128, H_CHUNK], bf16, name="in_stack")
                for q, t in enumerate(raws):
                    nc.gpsimd.dma_start(
                        out=in_stack[32 * q : 32 * (q + 1), :],
                        in_=t[:, h0 : h0 + H_CHUNK],
                    )

                # p2 stack: partition 32q+c = hpo[c, h0+512s+128q+j], cols (s j)
                p2_stack = p2sp.tile([128, H_CHUNK // 4], f32, name="p2_stack")
                for q in range(4):
                    nc.sync.dma_start(
                        out=p2_stack[32 * q : 32 * (q + 1), :],
                        in_=bass.AP(
                            hpo,
                            h0 + 128 * q,
                            [[H, C], [512, H_CHUNK // 512], [1, 128]],
                        ),
                    )

                # p2^2 -> bf16, plus exact f32 per-partition row-sum partial
                p2sq = p2sp.tile([128, H_CHUNK // 4], bf16, name="p2sq")
                ppart = p2sp.tile([128, 1], f32, name="ppart", bufs=N_CHUNKS)
                nc.scalar.activation(p2sq, p2_stack, Act.Square, accum_out=ppart)
                ppart_tiles.append(ppart)

                # transpose p2sq stacks -> p2t (h-part, c-free per 32-col block)
                p2t_ps = psp2p.tile([128, H_CHUNK // 4], bf16, name="p2t_ps")
                for s in range(H_CHUNK // 512):
                    nc.tensor.transpose(
                        p2t_ps[:, 128 * s : 128 * (s + 1)],
                        p2sq[:, 128 * s : 128 * (s + 1)],
                        ident_bf,
                    )
                p2t = p2sp.tile([128, H_CHUNK // 4], bf16, name="p2t")
                nc.scalar.copy(p2t, p2t_ps)

                # moving buffer: per h-tile t, cols 256t:256t+128 raw T,
                # 256t+128:256t+256 products [P1|P3|P2|P4]
                mvbuf = mvp.tile([128, 2 * H_CHUNK], bf16, name="mvbuf")

                for qd in range(QUADS_PER_CHUNK):
                    quad = pstp.tile([128, 1024], bf16, name="quad")
                    for j in range(8):
                        t = 8 * qd + j
                        nc.tensor.transpose(
                            quad[:, 128 * j : 128 * (j + 1)],
                            in_stack[:, 128 * t : 128 * (t + 1)],
                            ident_bf,
                        )
                    mv0 = 2048 * qd  # mvbuf col offset of this quad (8 tiles)
                    # raw copy: 8 tiles -> even 128-col slots
                    nc.scalar.copy(
                        bass.AP(
                            mvbuf.tensor,
                            mvbuf.offset + mv0,
                            [mvbuf.ap[0], [256, 8], [1, 128]],
                        ),
                        quad.rearrange("p (t f) -> p t f", f=128),
                    )
                # products from the SBUF raw copies, 16 h-tiles per op:
                # [P1|P3|P2|P4] <- [gh1|h1] x gh2, [gh1|h1] x h2
                for half in range(2):
                    base = mvbuf.offset + 4096 * half
                    for dst_off, a_off, b_off in (
                        (128, 0, 64),    # P1 = gh1*gh2
                        (160, 32, 64),   # P3 = h1*gh2
                        (192, 0, 96),    # P2 = gh1*h2
                        (224, 32, 96),   # P4 = h1*h2
                    ):
                        nc.vector.scalar_tensor_tensor(
                            bass.AP(mvbuf.tensor, base + dst_off,
                                    [mvbuf.ap[0], [256, 16], [1, 32]]),
                            in0=bass.AP(mvbuf.tensor, base + a_off,
                                        [mvbuf.ap[0], [256, 16], [1, 32]]),
                            scalar=1.0,
                            in1=bass.AP(mvbuf.tensor, base + b_off,
                                        [mvbuf.ap[0], [256, 16], [1, 32]]),
                            op0=Alu.bypass,
                            op1=Alu.mult,
                        )

                # mains: one matmul per h-tile, N=256
                for t in range(TILES_PER_CHUNK):
                    s, q = t // 4, t % 4
                    first = ch == 0 and t == 0
                    last = ch == N_CHUNKS - 1 and t == TILES_PER_CHUNK - 1
                    nc.tensor.matmul(
                        psum_main,
                        lhsT=p2t[:, 128 * s + 32 * q : 128 * s + 32 * (q + 1)],
                        rhs=mvbuf[:, 256 * t : 256 * (t + 1)],
                        start=first,
                        stop=last,
                    )

            # ---- finalize ----
            # P[c] = sum over q-blocks of ppart partials, via selection-matrix
            # matmul (DVE can't mix base partitions): S[32q+c, c'] = (c==c')
            acc = ppart_tiles[0]
            for i in range(1, N_CHUNKS):
                nxt = smallp.tile([128, 1], f32, name=f"pacc{i}")
                nc.vector.scalar_tensor_tensor(
                    nxt, in0=acc, scalar=1.0, in1=ppart_tiles[i],
                    op0=Alu.bypass, op1=Alu.add,
                )
                acc = nxt
            psum_vec = smallp.tile([C, 1], f32)
            nc.vector.memset(psum_vec, 0.0)
            for q in range(4):
                nc.gpsimd.dma_start(
                    out=psum_vec,
                    in_=acc[C * q : C * (q + 1), :],
                    accum_op=Alu.add,
                )
            pk2 = mul2("pk2", psum_vec, cK2)

            # copy psum_main to SBUF once (finalize ops need SBUF operands)
            main_sb = smallp.tile([C, 256], f32)
            nc.scalar.copy(main_sb, psum_main)

            # x3 = cA2*M1 + cAB*(M2+M3) + cB2*M4 + cAK*(Vg1+Vg2) + cBK*(Vh1+Vh2) + pk2
            # psum_main cols: 0:32 Vg1, 32:64 Vh1, 64:96 Vg2, 96:128 Vh2,
            #                 128:160 M1, 160:192 M3, 192:224 M2, 224:256 M4
            BC = B_CORE
            vg = smallp.tile([C, BC], f32)
            nc.vector.scalar_tensor_tensor(
                vg, in0=main_sb[:, 0:32], scalar=1.0, in1=main_sb[:, 64:96],
                op0=Alu.bypass, op1=Alu.add,
            )
            vh = smallp.tile([C, BC], f32)
            nc.vector.scalar_tensor_tensor(
                vh, in0=main_sb[:, 32:64], scalar=1.0, in1=main_sb[:, 96:128],
                op0=Alu.bypass, op1=Alu.add,
            )
            m23 = smallp.tile([C, BC], f32)
            nc.vector.scalar_tensor_tensor(
                m23, in0=main_sb[:, 160:192], scalar=1.0, in1=main_sb[:, 192:224],
                op0=Alu.bypass, op1=Alu.add,
            )
            e = smallp.tile([C, BC], f32, name="e1")
            nc.vector.tensor_scalar(
                e, in0=main_sb[:, 128:160], scalar1=cA2, scalar2=pk2,
                op0=Alu.mult, op1=Alu.add,
            )
            for nm, term, cf in (
                ("e2", m23, cAB),
                ("e3", main_sb[:, 224:256], cB2),
                ("e4", vg, cAK),
                ("e5", vh, cBK),
            ):
                e2 = smallp.tile([C, BC], f32, name=nm)
                nc.vector.scalar_tensor_tensor(
                    e2, in0=term, scalar=cf, in1=e, op0=Alu.mult, op1=Alu.add
                )
                e = e2

            # out = sqrt(x3)*out_scale + out_bias
            sq = smallp.tile([C, BC], f32)
            nc.scalar.activation(sq, e, Act.Sqrt)
            ocb = smallp.tile([C, BC], f32)
            nc.vector.tensor_scalar(
                ocb, in0=sq, scalar1=vec_sb[:, 0:1], scalar2=vec_sb[:, 1:2],
                op0=Alu.mult, op1=Alu.add,
            )

            # MLP head
            ps_h = psmlpp.tile([2 * C, BC], f32, tag="mlp")
            nc.tensor.matmul(ps_h, lhsT=w1t_sb, rhs=ocb, start=True, stop=True)
            hb = smallp.tile([2 * C, BC], f32)
            nc.scalar.activation(hb, ps_h, Act.Identity, bias=b1_sb, scale=1.0)
            hmin = smallp.tile([2 * C, BC], f32)
            nc.vector.tensor_scalar(hmin, in0=hb, scalar1=0.0, scalar2=None, op0=Alu.min)
            hmax = smallp.tile([2 * C, BC], f32)
            nc.vector.tensor_scalar(hmax, in0=hb, scalar1=0.0, scalar2=None, op0=Alu.max)
            hsb = smallp.tile([2 * C, BC], f32)
            nc.vector.scalar_tensor_tensor(
                hsb, in0=hmin, scalar=0.1, in1=hmax, op0=Alu.mult, op1=Alu.add
            )
            ps_z = psmlpp.tile([BC, 1], f32, tag="mlp")
            nc.tensor.matmul(ps_z, lhsT=hsb, rhs=w2_sb, start=True, stop=True)
            zf = smallp.tile([BC, 1], f32)
            nc.scalar.activation(zf, ps_z, Act.Sigmoid, bias=b2_sb, scale=1.0)
            nc.sync.dma_start(out=out[:, :], in_=zf)

    nc.compile()
    return nc


def _get_nc():
    if "nc" not in _cache:
        _cache["nc"] = _build()
    return _cache["nc"]


def kernel(
    gh1, h1, gh2, h2, hpo_par, out_scale, out_bias,
    genes_hpo_scale, hpo_bias, gen_bias, W1, b1, W2, b2,
):
    from concourse.bass_utils import run_bass_kernel_spmd

    nc = _get_nc()
    vec = np.zeros((10, C), np.float32)
    vec[0] = out_scale
    vec[1] = out_bias
    vec[2] = genes_hpo_scale
    vec[3] = hpo_bias
    vec[4] = gen_bias
    vec[5:7] = np.asarray(b1, np.float32).reshape(2, C)
    vec[7, 0] = np.float32(np.asarray(b2).reshape(-1)[0])
    vec[8:10] = np.asarray(W2, np.float32).reshape(2, C)

    gh1 = np.ascontiguousarray(gh1, np.float32)
    h1 = np.ascontiguousarray(h1, np.float32)
    gh2 = np.ascontiguousarray(gh2, np.float32)
    h2 = np.ascontiguousarray(h2, np.float32)
    hpo = np.ascontiguousarray(hpo_par, np.float32)
    w1 = np.ascontiguousarray(W1, np.float32)

    in_maps = []
    for c in range(N_CORES):
        sl = slice(c * B_CORE, (c + 1) * B_CORE)
        in_maps.append(
            {
                "gh1": gh1[sl], "h1": h1[sl], "gh2": gh2[sl], "h2": h2[sl],
                "hpo": hpo, "w1": w1, "vec": vec,
            }
        )

    res = run_bass_kernel_spmd(nc, in_maps, core_ids=list(range(N_CORES)))
    outs = [res.results[c]["out"] for c in range(N_CORES)]
    return np.concatenate(outs, axis=0).astype(np.float32)


# revision 28
# speedup vs baseline: 1.8014x; 1.8014x over previous
"""Trainium2 Bass kernel for nn_PhenoConnect7 (dense_mlp, memory-bound).

Math: the reference computes, per batch row b and channel c (C=32, H=16384):
    x1 = A_c*gh1 + B_c*h1 + K_c        (A = s, B = s*exp(gen_bias),
    x2 = A_c*gh2 + B_c*h2 + K_c         K = G_LEN*s*exp(hpo_bias)*(1+exp(gen_bias)))
    x3[b,c] = sum_h x1*x2*p2[c,h]       (p2 = hpo_par^2)
    out = sqrt(x3)*out_scale + out_bias ; tiny MLP head -> sigmoid -> (B,1)

Expanding x1*x2 turns x3 into 8 matmuls against p2 plus a per-channel constant:
    x3 = A^2*M11 + A*B*(M12+M21) + B^2*M22
       + A*K*(Vg1+Vg2) + B*K*(Vh1+Vh2) + K^2*P
with M.. = sum_h (prod of two inputs)*p2, V.. = sum_h input*p2, P = sum_h p2.

Kernel strategy (pure batch-parallel across 8 cores, 32 rows each):
  - DMA inputs f32->bf16 (SWDGE cast) into a 4-tensor partition stack
    (128 part = 4 tensors x 32 batch rows, free = H chunk).
  - PE transposes 128x128 stack tiles -> h-partitioned tiles in PSUM.
  - ACT copies raw transposed tiles PSUM->SBUF; DVE forms the 4 pairwise
    products directly from PSUM into SBUF (interleaved [raw|prod] layout).
  - p2 = hpo_par^2 computed on DVE (bf16 out + exact f32 row-sum partials for
    the dominant K^2*P term), PE-transposed to p2T (h-part, c-free).
  - Mains: per 128-h tile one matmul, stationary p2T (128,32), moving the
    256-col [rawT|prodT] stack; PSUM (32c, 256) accumulates all 9 sums.
  - Finalize on (32,32): combine with per-channel coeffs, sqrt, MLP head,
    sigmoid, DMA out (32,1) per core.
"""

import os
import numpy as np

B_FULL = 256
H = 16384
C = 32
N_CORES = 8
B_CORE = B_FULL // N_CORES  # 32
G_LEN = 20000.0

H_CHUNK = 4096
N_CHUNKS = H // H_CHUNK           # 4
QTR = H_CHUNK // 4                # 1024: quarter of a chunk
TPQ = QTR // 128                  # 8 transpose ops per (tensor, chunk)
SS = 8 * 128                      # mvbuf super-slot: 4 raw + 4 product blocks

_cache = {}


def _build():
    import concourse.bass as bass
    import concourse.mybir as mybir
    from concourse import bacc
    from concourse.tile import TileContext
    from concourse.masks import make_identity

    f32 = mybir.dt.float32
    bf16 = mybir.dt.bfloat16
    Alu = mybir.AluOpType
    Act = mybir.ActivationFunctionType

    nc = bacc.Bacc()

    gh1 = nc.dram_tensor("gh1", [B_CORE, H], f32, kind="ExternalInput")
    h1 = nc.dram_tensor("h1", [B_CORE, H], f32, kind="ExternalInput")
    gh2 = nc.dram_tensor("gh2", [B_CORE, H], f32, kind="ExternalInput")
    h2 = nc.dram_tensor("h2", [B_CORE, H], f32, kind="ExternalInput")
    hpo = nc.dram_tensor("hpo", [C, H], f32, kind="ExternalInput")
    w1 = nc.dram_tensor("w1", [2 * C, C], f32, kind="ExternalInput")
    # vec rows: 0 out_scale, 1 out_bias, 2 genes_hpo_scale, 3 hpo_bias,
    # 4 gen_bias, 5-6 b1, 7 [b2,...], 8-9 W2
    vec = nc.dram_tensor("vec", [10, C], f32, kind="ExternalInput")
    out = nc.dram_tensor("out", [B_CORE, 1], f32, kind="ExternalOutput")

    raws = [gh1, h1, gh2, h2]

    with TileContext(nc) as tc:
        with (
            tc.tile_pool(name="const", bufs=1) as constp,
            tc.tile_pool(name="stack", bufs=3) as stackp,
            tc.tile_pool(name="p2s", bufs=3) as p2sp,
            tc.tile_pool(name="mv", bufs=2) as mvp,
            tc.tile_pool(name="small", bufs=1) as smallp,
            tc.tile_pool(name="pst", bufs=3, space="PSUM") as pstp,
            tc.tile_pool(name="psp2", bufs=2, space="PSUM") as psp2p,
            tc.tile_pool(name="psmain", bufs=1, space="PSUM") as psmainp,
            tc.tile_pool(name="psmlp", bufs=1, space="PSUM") as psmlpp,
        ):
            ident_bf = constp.tile([128, 128], bf16)
            make_identity(nc, ident_bf)

            # ---- small parameter loads ----
            vec_sb = smallp.tile([C, 10], f32)  # vec transposed: col r = vec row r
            nc.sync.dma_start(
                out=vec_sb, in_=bass.AP(vec, 0, [[1, C], [C, 10]])
            )
            b1_sb = smallp.tile([2 * C, 1], f32)
            nc.sync.dma_start(out=b1_sb, in_=bass.AP(vec, 5 * C, [[1, 2 * C], [1, 1]]))
            w2_sb = smallp.tile([2 * C, 1], f32)
            nc.sync.dma_start(out=w2_sb, in_=bass.AP(vec, 8 * C, [[1, 2 * C], [1, 1]]))
            b2_row = smallp.tile([1, 1], f32)
            nc.sync.dma_start(out=b2_row, in_=bass.AP(vec, 7 * C, [[1, 1], [1, 1]]))
            b2_sb = smallp.tile([C, 1], f32)
            nc.gpsimd.partition_broadcast(b2_sb, b2_row)

            # W1T (32,64) loaded directly with a strided (transposing) DMA --
            # tiny tensor, avoids an f32 PE matmul (1-wait codegen limit)
            w1t_sb = smallp.tile([C, 2 * C], f32)
            nc.sync.dma_start(out=w1t_sb, in_=bass.AP(w1, 0, [[1, C], [C, 2 * C]]))

            # ---- per-channel coefficients (32,1 each) ----
            s_ap = vec_sb[:, 2:3]
            eg = smallp.tile([C, 1], f32)
            nc.scalar.activation(eg, vec_sb[:, 4:5], Act.Exp)
            eh = smallp.tile([C, 1], f32)
            nc.scalar.activation(eh, vec_sb[:, 3:4], Act.Exp)
            coefB = smallp.tile([C, 1], f32)
            nc.vector.scalar_tensor_tensor(
                coefB, in0=s_ap, scalar=1.0, in1=eg, op0=Alu.bypass, op1=Alu.mult
            )
            # K = G_LEN * s * eh * (1+eg)
            t1 = smallp.tile([C, 1], f32)
            nc.vector.scalar_tensor_tensor(
                t1, in0=eg, scalar=1.0, in1=eh, op0=Alu.add, op1=Alu.mult
            )
            coefK = smallp.tile([C, 1], f32)
            nc.vector.scalar_tensor_tensor(
                coefK, in0=t1, scalar=G_LEN, in1=s_ap, op0=Alu.mult, op1=Alu.mult
            )

            def mul2(name, a, b):
                t = smallp.tile([C, 1], f32, name=name)
                nc.vector.scalar_tensor_tensor(
                    t, in0=a, scalar=1.0, in1=b, op0=Alu.bypass, op1=Alu.mult
                )
                return t

            cA2 = mul2("cA2", s_ap, s_ap)
            cAB = mul2("cAB", s_ap, coefB)
            cB2 = mul2("cB2", coefB, coefB)
            cAK = mul2("cAK", s_ap, coefK)
            cBK = mul2("cBK", coefB, coefK)
            cK2 = mul2("cK2", coefK, coefK)

            # ---- main loop over H chunks ----
            # psum_main blocks (32 cols each):
            # [Vg1 | Vh1 | Vg2 | Vh2 | M1 | M3 | M2 | M4]
            psum_main = psmainp.tile([C, 256], f32)
            ppart_tiles = []

            for ch in range(N_CHUNKS):
                h0 = ch * H_CHUNK

                # quarter-stacked loads: partition 32a+r = src[r, h0 + QTR*a + j]
                # (full 128-partition DMAs, 3-dim APs, 4KB contiguous reads)
                in_stacks = []
                for q, t in enumerate(raws):
                    st = stackp.tile([128, QTR], bf16, name=f"in_stack{q}")
                    nc.gpsimd.dma_start(
                        out=st,
                        in_=bass.AP(t, h0, [[QTR, 4], [H, B_CORE], [1, QTR]]),
                    )
                    in_stacks.append(st)

                p2_stack = p2sp.tile([128, QTR], f32, name="p2_stack")
                nc.sync.dma_start(
                    out=p2_stack,
                    in_=bass.AP(hpo, h0, [[QTR, 4], [H, C], [1, QTR]]),
                )

                # p2^2 -> bf16, plus exact f32 per-partition row-sum partial
                p2sq = p2sp.tile([128, QTR], bf16, name="p2sq")
                ppart = p2sp.tile([128, 1], f32, name="ppart", bufs=N_CHUNKS)
                nc.scalar.activation(p2sq, p2_stack, Act.Square, accum_out=ppart)
                ppart_tiles.append(ppart)

                # transpose p2sq: op tp covers h-tiles (k, tp) at col-block k
                p2t = p2sp.tile([128, QTR], bf16, name="p2t")
                p2t_ps = psp2p.tile([128, 1024], bf16, name="p2t_ps")
                for tp in range(TPQ):
                    nc.tensor.transpose(
                        p2t_ps[:, 128 * tp : 128 * (tp + 1)],
                        p2sq[:, 128 * tp : 128 * (tp + 1)],
                        ident_bf,
                    )
                nc.scalar.copy(p2t, p2t_ps)

                # moving buffer: per tp, a super-slot of SS cols (128 each):
                # [gh1T|h1T|gh2T|h2T | P1|Pab|P4|g|h | a|b(scratch)]
                mvbuf = mvp.tile([128, TPQ * SS], bf16, name="mvbuf")

                for q in range(4):
                    ps_q = pstp.tile([128, 1024], bf16, name="ps_q", bufs=4)
                    for tp in range(TPQ):
                        nc.tensor.transpose(
                            ps_q[:, 128 * tp : 128 * (tp + 1)],
                            in_stacks[q][:, 128 * tp : 128 * (tp + 1)],
                            ident_bf,
                        )
                    dst = bass.AP(
                        mvbuf.tensor,
                        mvbuf.offset + 128 * q,
                        [mvbuf.ap[0], [SS, 8], [1, 128]],
                    )
                    srcv = ps_q.rearrange("p (t f) -> p t f", f=128)
                    if q % 2 == 0:
                        nc.scalar.copy(dst, srcv)
                    else:
                        nc.vector.tensor_copy(dst, srcv)

                # products (SBUF->SBUF, one op per product per chunk):
                def mvap(off):
                    return bass.AP(
                        mvbuf.tensor,
                        mvbuf.offset + off,
                        [mvbuf.ap[0], [SS, TPQ], [1, 128]],
                    )

                for dst_off, a_off, b_off in (
                    (512, 0, 256),    # P1 = gh1*gh2
                    (640, 128, 256),  # P3 = h1*gh2
                    (768, 0, 384),    # P2 = gh1*h2
                    (896, 128, 384),  # P4 = h1*h2
                ):
                    nc.vector.scalar_tensor_tensor(
                        mvap(dst_off), in0=mvap(a_off), scalar=1.0,
                        in1=mvap(b_off), op0=Alu.bypass, op1=Alu.mult,
                    )

                # mains: h-tile (k, tp): lhsT = p2t col 128*tp+32k; rhs = all
                # 8 blocks (raw q0..3, P1, P3, P2, P4), 32 cols each, stride 128
                for tp in range(TPQ):
                    for k in range(4):
                        first = ch == 0 and tp == 0 and k == 0
                        last = ch == N_CHUNKS - 1 and tp == TPQ - 1 and k == 3
                        nc.tensor.matmul(
                            psum_main,
                            lhsT=p2t[:, 128 * tp + 32 * k : 128 * tp + 32 * (k + 1)],
                            rhs=bass.AP(
                                mvbuf.tensor,
                                mvbuf.offset + SS * tp + 32 * k,
                                [mvbuf.ap[0], [128, 8], [1, 32]],
                            ),
                            start=first,
                            stop=last,
                        )

            # ---- finalize ----
            # P[c] = sum over q-blocks of ppart partials, via selection-matrix
            # matmul (DVE can't mix base partitions): S[32q+c, c'] = (c==c')
            acc = ppart_tiles[0]
            for i in range(1, N_CHUNKS):
                nxt = smallp.tile([128, 1], f32, name=f"pacc{i}")
                nc.vector.scalar_tensor_tensor(
                    nxt, in0=acc, scalar=1.0, in1=ppart_tiles[i],
                    op0=Alu.bypass, op1=Alu.add,
                )
                acc = nxt
            psum_vec = smallp.tile([C, 1], f32)
            nc.vector.memset(psum_vec, 0.0)
            for q in range(4):
                nc.gpsimd.dma_start(
                    out=psum_vec,
                    in_=acc[C * q : C * (q + 1), :],
                    accum_op=Alu.add,
                )
            pk2 = mul2("pk2", psum_vec, cK2)

            # copy psum_main to SBUF once (finalize ops need SBUF operands)
            main_sb = smallp.tile([C, 256], f32)
            nc.scalar.copy(main_sb, psum_main)

            # x3 = cA2*M1 + cAB*(M2+M3) + cB2*M4 + cAK*(Vg1+Vg2) + cBK*(Vh1+Vh2)+pk2
            # cols: 0:32 Vg1, 32:64 Vh1, 64:96 Vg2, 96:128 Vh2,
            #       128:160 M1, 160:192 M3, 192:224 M2, 224:256 M4
            BC = B_CORE
            vg = smallp.tile([C, BC], f32)
            nc.vector.scalar_tensor_tensor(
                vg, in0=main_sb[:, 0:32], scalar=1.0, in1=main_sb[:, 64:96],
                op0=Alu.bypass, op1=Alu.add,
            )
            vh = smallp.tile([C, BC], f32)
            nc.vector.scalar_tensor_tensor(
                vh, in0=main_sb[:, 32:64], scalar=1.0, in1=main_sb[:, 96:128],
                op0=Alu.bypass, op1=Alu.add,
            )
            m23 = smallp.tile([C, BC], f32)
            nc.vector.scalar_tensor_tensor(
                m23, in0=main_sb[:, 160:192], scalar=1.0, in1=main_sb[:, 192:224],
                op0=Alu.bypass, op1=Alu.add,
            )
            e = smallp.tile([C, BC], f32, name="e1")
            nc.vector.tensor_scalar(
                e, in0=main_sb[:, 128:160], scalar1=cA2, scalar2=pk2,
                op0=Alu.mult, op1=Alu.add,
            )
            for nm, term, cf in (
                ("e2", m23, cAB),
                ("e3", main_sb[:, 224:256], cB2),
                ("e4", vg, cAK),
                ("e5", vh, cBK),
            ):
                e2 = smallp.tile([C, BC], f32, name=nm)
                nc.vector.scalar_tensor_tensor(
                    e2, in0=term, scalar=cf, in1=e, op0=Alu.mult, op1=Alu.add
                )
                e = e2

            # out = sqrt(x3)*out_scale + out_bias
            sq = smallp.tile([C, BC], f32)
            nc.scalar.activation(sq, e, Act.Sqrt)
            ocb = smallp.tile([C, BC], f32)
            nc.vector.tensor_scalar(
                ocb, in0=sq, scalar1=vec_sb[:, 0:1], scalar2=vec_sb[:, 1:2],
                op0=Alu.mult, op1=Alu.add,
            )

            # MLP head
            ps_h = psmlpp.tile([2 * C, BC], f32, tag="mlp")
            nc.tensor.matmul(ps_h, lhsT=w1t_sb, rhs=ocb, start=True, stop=True)
            hb = smallp.tile([2 * C, BC], f32)
            nc.scalar.activation(hb, ps_h, Act.Identity, bias=b1_sb, scale=1.0)
            hmin = smallp.tile([2 * C, BC], f32)
            nc.vector.tensor_scalar(hmin, in0=hb, scalar1=0.0, scalar2=None, op0=Alu.min)
            hmax = smallp.tile([2 * C, BC], f32)
            nc.vector.tensor_scalar(hmax, in0=hb, scalar1=0.0, scalar2=None, op0=Alu.max)
            hsb = smallp.tile([2 * C, BC], f32)
            nc.vector.scalar_tensor_tensor(
                hsb, in0=hmin, scalar=0.1, in1=hmax, op0=Alu.mult, op1=Alu.add
            )
            ps_z = psmlpp.tile([BC, 1], f32, tag="mlp")
            nc.tensor.matmul(ps_z, lhsT=hsb, rhs=w2_sb, start=True, stop=True)
            zf = smallp.tile([BC, 1], f32)
            nc.scalar.activation(zf, ps_z, Act.Sigmoid, bias=b2_sb, scale=1.0)
            nc.sync.dma_start(out=out[:, :], in_=zf)

    nc.compile()
    return nc


def _get_nc():
    if "nc" not in _cache:
        _cache["nc"] = _build()
    return _cache["nc"]


def kernel(
    gh1, h1, gh2, h2, hpo_par, out_scale, out_bias,
    genes_hpo_scale, hpo_bias, gen_bias, W1, b1, W2, b2,
):
    from concourse.bass_utils import run_bass_kernel_spmd

    nc = _get_nc()
    vec = np.zeros((10, C), np.float32)
    vec[0] = out_scale
    vec[1] = out_bias
    vec[2] = genes_hpo_scale
    vec[3] = hpo_bias
    vec[4] = gen_bias
    vec[5:7] = np.asarray(b1, np.float32).reshape(2, C)
    vec[7, 0] = np.float32(np.asarray(b2).reshape(-1)[0])
    vec[8:10] = np.asarray(W2, np.float32).reshape(2, C)

    gh1 = np.ascontiguousarray(gh1, np.float32)
    h1 = np.ascontiguousarray(h1, np.float32)
    gh2 = np.ascontiguousarray(gh2, np.float32)
    h2 = np.ascontiguousarray(h2, np.float32)
    hpo = np.ascontiguousarray(hpo_par, np.float32)
    w1 = np.ascontiguousarray(W1, np.float32)

    in_maps = []
    for c in range(N_CORES):
        sl = slice(c * B_CORE, (c + 1) * B_CORE)
        in_maps.append(
            {
                "gh1": gh1[sl], "h1": h1[sl], "gh2": gh2[sl], "h2": h2[sl],
                "hpo": hpo, "w1": w1, "vec": vec,
            }
        )

    res = run_bass_kernel_spmd(nc, in_maps, core_ids=list(range(N_CORES)))
    outs = [res.results[c]["out"] for c in range(N_CORES)]
    return np.concatenate(outs, axis=0).astype(np.float32)


# revision 32
# speedup vs baseline: 1.8366x; 1.0195x over previous
"""Trainium2 Bass kernel for nn_PhenoConnect7 (dense_mlp, memory-bound).

Math: the reference computes, per batch row b and channel c (C=32, H=16384):
    x1 = A_c*gh1 + B_c*h1 + K_c        (A = s, B = s*exp(gen_bias),
    x2 = A_c*gh2 + B_c*h2 + K_c         K = G_LEN*s*exp(hpo_bias)*(1+exp(gen_bias)))
    x3[b,c] = sum_h x1*x2*p2[c,h]       (p2 = hpo_par^2)
    out = sqrt(x3)*out_scale + out_bias ; tiny MLP head -> sigmoid -> (B,1)

Expanding x1*x2 turns x3 into 8 matmuls against p2 plus a per-channel constant:
    x3 = A^2*M11 + A*B*(M12+M21) + B^2*M22
       + A*K*(Vg1+Vg2) + B*K*(Vh1+Vh2) + K^2*P
with M.. = sum_h (prod of two inputs)*p2, V.. = sum_h input*p2, P = sum_h p2.

Kernel strategy (pure batch-parallel across 8 cores, 32 rows each):
  - Quarter-stacked loads: per tensor per 4096-H chunk, one SWDGE DMA with
    inline f32->bf16 cast into (128 part = 4 H-quarters x 32 rows, 1024 free)
    -- full-port 128-partition writes, 3-dim APs, 4KB contiguous reads.
  - PE transposes each 128x128 tile; a transpose of the quarter-stack yields
    4 h-tiles at once (col-block a = quarter a's h-tile).
  - One batched ACT/DVE copy per tensor moves transposed tiles PSUM->SBUF
    into per-h-tile super-slots [gh1T|h1T|gh2T|h2T|P1|P3|P2|P4] (128 cols
    each); DVE forms the 4 pairwise products SBUF->SBUF (bf16 4x mode).
  - p2 = hpo_par^2 on ACT (Square, bf16 out + exact f32 accum_out partials
    for the dominant K^2*P term), PE-transposed to p2T via the same
    quarter-stack trick; P folded exactly with 4 tiny accumulate-DMAs.
  - Mains: one matmul per 128-h tile: stationary p2T (128,32), moving the
    8 blocks (strided AP, N=256); PSUM (32c, 256) accumulates all 8 sums
    over the full contraction (f32).
  - Finalize on (32,32): per-channel coeff combine (DVE), sqrt (ACT), MLP
    head (f32 matmuls), sigmoid, DMA out (32,1) per core.

Notes from bring-up: tensor_tensor_reduce crashes the device (use ACT
Square+accum_out instead); f32 matmuls can carry only one sync wait (walrus
LW-struct limit) so W1T is DMA-transposed and the P-fold uses accum-DMAs;
Bacc.compile() is required for move_matmul_waits_to_ldweights +
generate_event_semaphores.
"""

import os
import numpy as np

B_FULL = 256
H = 16384
C = 32
N_CORES = 8
B_CORE = B_FULL // N_CORES  # 32
G_LEN = 20000.0

H_CHUNK = 4096
N_CHUNKS = H // H_CHUNK           # 4
QTR = H_CHUNK // 4                # 1024: quarter of a chunk
TPQ = QTR // 128                  # 8 transpose ops per (tensor, chunk)
SS = 8 * 128                      # mvbuf super-slot: 4 raw + 4 product blocks

_cache = {}


def _build():
    import concourse.bass as bass
    import concourse.mybir as mybir
    from concourse import bacc
    from concourse.tile import TileContext
    from concourse.masks import make_identity

    f32 = mybir.dt.float32
    bf16 = mybir.dt.bfloat16
    Alu = mybir.AluOpType
    Act = mybir.ActivationFunctionType

    nc = bacc.Bacc()

    gh1 = nc.dram_tensor("gh1", [B_CORE, H], f32, kind="ExternalInput")
    h1 = nc.dram_tensor("h1", [B_CORE, H], f32, kind="ExternalInput")
    gh2 = nc.dram_tensor("gh2", [B_CORE, H], f32, kind="ExternalInput")
    h2 = nc.dram_tensor("h2", [B_CORE, H], f32, kind="ExternalInput")
    hpo = nc.dram_tensor("hpo", [C, H], f32, kind="ExternalInput")
    w1 = nc.dram_tensor("w1", [2 * C, C], f32, kind="ExternalInput")
    # vec rows: 0 out_scale, 1 out_bias, 2 genes_hpo_scale, 3 hpo_bias,
    # 4 gen_bias, 5-6 b1, 7 [b2,...], 8-9 W2
    vec = nc.dram_tensor("vec", [10, C], f32, kind="ExternalInput")
    out = nc.dram_tensor("out", [B_CORE, 1], f32, kind="ExternalOutput")

    raws = [gh1, h1, gh2, h2]

    with TileContext(nc) as tc:
        with (
            tc.tile_pool(name="const", bufs=1) as constp,
            tc.tile_pool(name="stack", bufs=3) as stackp,
            tc.tile_pool(name="p2s", bufs=3) as p2sp,
            tc.tile_pool(name="mv", bufs=2) as mvp,
            tc.tile_pool(name="small", bufs=1) as smallp,
            tc.tile_pool(name="pst", bufs=3, space="PSUM") as pstp,
            tc.tile_pool(name="psp2", bufs=2, space="PSUM") as psp2p,
            tc.tile_pool(name="psmain", bufs=1, space="PSUM") as psmainp,
            tc.tile_pool(name="psmlp", bufs=1, space="PSUM") as psmlpp,
        ):
            ident_bf = constp.tile([128, 128], bf16)
            make_identity(nc, ident_bf)

            # ---- small parameter loads ----
            vec_sb = smallp.tile([C, 10], f32)  # vec transposed: col r = vec row r
            nc.sync.dma_start(
                out=vec_sb, in_=bass.AP(vec, 0, [[1, C], [C, 10]])
            )
            b1_sb = smallp.tile([2 * C, 1], f32)
            nc.sync.dma_start(out=b1_sb, in_=bass.AP(vec, 5 * C, [[1, 2 * C], [1, 1]]))
            w2_sb = smallp.tile([2 * C, 1], f32)
            nc.sync.dma_start(out=w2_sb, in_=bass.AP(vec, 8 * C, [[1, 2 * C], [1, 1]]))
            b2_row = smallp.tile([1, 1], f32)
            nc.sync.dma_start(out=b2_row, in_=bass.AP(vec, 7 * C, [[1, 1], [1, 1]]))
            b2_sb = smallp.tile([C, 1], f32)
            nc.gpsimd.partition_broadcast(b2_sb, b2_row)

            # W1T (32,64) loaded directly with a strided (transposing) DMA --
            # tiny tensor, avoids an f32 PE matmul (1-wait codegen limit)
            w1t_sb = smallp.tile([C, 2 * C], f32)
            nc.sync.dma_start(out=w1t_sb, in_=bass.AP(w1, 0, [[1, C], [C, 2 * C]]))

            # ---- per-channel coefficients (32,1 each) ----
            s_ap = vec_sb[:, 2:3]
            eg = smallp.tile([C, 1], f32)
            nc.scalar.activation(eg, vec_sb[:, 4:5], Act.Exp)
            eh = smallp.tile([C, 1], f32)
            nc.scalar.activation(eh, vec_sb[:, 3:4], Act.Exp)
            coefB = smallp.tile([C, 1], f32)
            nc.vector.scalar_tensor_tensor(
                coefB, in0=s_ap, scalar=1.0, in1=eg, op0=Alu.bypass, op1=Alu.mult
            )
            # K = G_LEN * s * eh * (1+eg)
            t1 = smallp.tile([C, 1], f32)
            nc.vector.scalar_tensor_tensor(
                t1, in0=eg, scalar=1.0, in1=eh, op0=Alu.add, op1=Alu.mult
            )
            coefK = smallp.tile([C, 1], f32)
            nc.vector.scalar_tensor_tensor(
                coefK, in0=t1, scalar=G_LEN, in1=s_ap, op0=Alu.mult, op1=Alu.mult
            )

            def mul2(name, a, b):
                t = smallp.tile([C, 1], f32, name=name)
                nc.vector.scalar_tensor_tensor(
                    t, in0=a, scalar=1.0, in1=b, op0=Alu.bypass, op1=Alu.mult
                )
                return t

            cA2 = mul2("cA2", s_ap, s_ap)
            cAB = mul2("cAB", s_ap, coefB)
            cB2 = mul2("cB2", coefB, coefB)
            cAK = mul2("cAK", s_ap, coefK)
            cBK = mul2("cBK", coefB, coefK)
            cK2 = mul2("cK2", coefK, coefK)

            # ---- main loop over H chunks ----
            # psum_main blocks (32 cols each):
            # [Vg1 | Vh1 | Vg2 | Vh2 | M1 | M3 | M2 | M4]
            psum_main = psmainp.tile([C, 256], f32)
            ppart_tiles = []

            for ch in range(N_CHUNKS):
                h0 = ch * H_CHUNK

                # quarter-stacked loads: partition 32a+r = src[r, h0 + QTR*a + j]
                # (full 128-partition DMAs, 3-dim APs, 4KB contiguous reads)
                in_stacks = []
                for q, t in enumerate(raws):
                    st = stackp.tile([128, QTR], bf16, name=f"in_stack{q}")
                    nc.gpsimd.dma_start(
                        out=st,
                        in_=bass.AP(t, h0, [[QTR, 4], [H, B_CORE], [1, QTR]]),
                    )
                    in_stacks.append(st)

                p2_stack = p2sp.tile([128, QTR], f32, name="p2_stack")
                nc.sync.dma_start(
                    out=p2_stack,
                    in_=bass.AP(hpo, h0, [[QTR, 4], [H, C], [1, QTR]]),
                )

                # p2^2 -> bf16, plus exact f32 per-partition row-sum partial
                p2sq = p2sp.tile([128, QTR], bf16, name="p2sq")
                ppart = p2sp.tile([128, 1], f32, name="ppart", bufs=N_CHUNKS)
                nc.scalar.activation(p2sq, p2_stack, Act.Square, accum_out=ppart)
                ppart_tiles.append(ppart)

                # transpose p2sq: op tp covers h-tiles (k, tp) at col-block k
                p2t = p2sp.tile([128, QTR], bf16, name="p2t")
                p2t_ps = psp2p.tile([128, 1024], bf16, name="p2t_ps")
                for tp in range(TPQ):
                    nc.tensor.transpose(
                        p2t_ps[:, 128 * tp : 128 * (tp + 1)],
                        p2sq[:, 128 * tp : 128 * (tp + 1)],
                        ident_bf,
                    )
                nc.scalar.copy(p2t, p2t_ps)

                # moving buffer: per tp, a super-slot of SS cols (128 each):
                # [gh1T|h1T|gh2T|h2T | P1|Pab|P4|g|h | a|b(scratch)]
                mvbuf = mvp.tile([128, TPQ * SS], bf16, name="mvbuf")

                for q in range(4):
                    ps_q = pstp.tile([128, 1024], bf16, name="ps_q", bufs=4)
                    for tp in range(TPQ):
                        nc.tensor.transpose(
                            ps_q[:, 128 * tp : 128 * (tp + 1)],
                            in_stacks[q][:, 128 * tp : 128 * (tp + 1)],
                            ident_bf,
                        )
                    dst = bass.AP(
                        mvbuf.tensor,
                        mvbuf.offset + 128 * q,
                        [mvbuf.ap[0], [SS, 8], [1, 128]],
                    )
                    srcv = ps_q.rearrange("p (t f) -> p t f", f=128)
                    if q % 2 == 0:
                        nc.scalar.copy(dst, srcv)
                    else:
                        nc.vector.tensor_copy(dst, srcv)

                # products + mains in half-chunk groups so mains of the
                # first half overlap the second half's product ops
                def mvap(off, base, n):
                    return bass.AP(
                        mvbuf.tensor,
                        mvbuf.offset + base + off,
                        [mvbuf.ap[0], [SS, n], [1, 128]],
                    )

                HTP = TPQ // 2
                for half in range(2):
                    base = SS * HTP * half
                    for dst_off, a_off, b_off in (
                        (512, 0, 256),    # P1 = gh1*gh2
                        (640, 128, 256),  # P3 = h1*gh2
                        (768, 0, 384),    # P2 = gh1*h2
                        (896, 128, 384),  # P4 = h1*h2
                    ):
                        nc.vector.scalar_tensor_tensor(
                            mvap(dst_off, base, HTP),
                            in0=mvap(a_off, base, HTP), scalar=1.0,
                            in1=mvap(b_off, base, HTP),
                            op0=Alu.bypass, op1=Alu.mult,
                        )
                    for tp in range(HTP * half, HTP * (half + 1)):
                        for k in range(4):
                            first = ch == 0 and tp == 0 and k == 0
                            last = ch == N_CHUNKS - 1 and tp == TPQ - 1 and k == 3
                            nc.tensor.matmul(
                                psum_main,
                                lhsT=p2t[:, 128 * tp + 32 * k : 128 * tp + 32 * (k + 1)],
                                rhs=bass.AP(
                                    mvbuf.tensor,
                                    mvbuf.offset + SS * tp + 32 * k,
                                    [mvbuf.ap[0], [128, 8], [1, 32]],
                                ),
                                start=first,
                                stop=last,
                            )

            # ---- finalize ----
            # P[c] = sum over q-blocks of ppart partials, via selection-matrix
            # matmul (DVE can't mix base partitions): S[32q+c, c'] = (c==c')
            acc = ppart_tiles[0]
            for i in range(1, N_CHUNKS):
                nxt = smallp.tile([128, 1], f32, name=f"pacc{i}")
                nc.vector.scalar_tensor_tensor(
                    nxt, in0=acc, scalar=1.0, in1=ppart_tiles[i],
                    op0=Alu.bypass, op1=Alu.add,
                )
                acc = nxt
            psum_vec = smallp.tile([C, 1], f32)
            nc.vector.memset(psum_vec, 0.0)
            for q in range(4):
                nc.gpsimd.dma_start(
                    out=psum_vec,
                    in_=acc[C * q : C * (q + 1), :],
                    accum_op=Alu.add,
                )
            pk2 = mul2("pk2", psum_vec, cK2)

            # copy psum_main to SBUF once (finalize ops need SBUF operands)
            main_sb = smallp.tile([C, 256], f32)
            nc.scalar.copy(main_sb, psum_main)

            # x3 = cA2*M1 + cAB*(M2+M3) + cB2*M4 + cAK*(Vg1+Vg2) + cBK*(Vh1+Vh2)+pk2
            # cols: 0:32 Vg1, 32:64 Vh1, 64:96 Vg2, 96:128 Vh2,
            #       128:160 M1, 160:192 M3, 192:224 M2, 224:256 M4
            BC = B_CORE
            vg = smallp.tile([C, BC], f32)
            nc.vector.scalar_tensor_tensor(
                vg, in0=main_sb[:, 0:32], scalar=1.0, in1=main_sb[:, 64:96],
                op0=Alu.bypass, op1=Alu.add,
            )
            vh = smallp.tile([C, BC], f32)
            nc.vector.scalar_tensor_tensor(
                vh, in0=main_sb[:, 32:64], scalar=1.0, in1=main_sb[:, 96:128],
                op0=Alu.bypass, op1=Alu.add,
            )
            m23 = smallp.tile([C, BC], f32)
            nc.vector.scalar_tensor_tensor(
                m23, in0=main_sb[:, 160:192], scalar=1.0, in1=main_sb[:, 192:224],
                op0=Alu.bypass, op1=Alu.add,
            )
            e = smallp.tile([C, BC], f32, name="e1")
            nc.vector.tensor_scalar(
                e, in0=main_sb[:, 128:160], scalar1=cA2, scalar2=pk2,
                op0=Alu.mult, op1=Alu.add,
            )
            for nm, term, cf in (
                ("e2", m23, cAB),
                ("e3", main_sb[:, 224:256], cB2),
                ("e4", vg, cAK),
                ("e5", vh, cBK),
            ):
                e2 = smallp.tile([C, BC], f32, name=nm)
                nc.vector.scalar_tensor_tensor(
                    e2, in0=term, scalar=cf, in1=e, op0=Alu.mult, op1=Alu.add
                )
                e = e2

            # out = sqrt(x3)*out_scale + out_bias
            sq = smallp.tile([C, BC], f32)
            nc.scalar.activation(sq, e, Act.Sqrt)
            ocb = smallp.tile([C, BC], f32)
            nc.vector.tensor_scalar(
                ocb, in0=sq, scalar1=vec_sb[:, 0:1], scalar2=vec_sb[:, 1:2],
                op0=Alu.mult, op1=Alu.add,
            )

            # MLP head
            ps_h = psmlpp.tile([2 * C, BC], f32, tag="mlp")
            nc.tensor.matmul(ps_h, lhsT=w1t_sb, rhs=ocb, start=True, stop=True)
            hb = smallp.tile([2 * C, BC], f32)
            nc.scalar.activation(hb, ps_h, Act.Identity, bias=b1_sb, scale=1.0)
            hmin = smallp.tile([2 * C, BC], f32)
            nc.vector.tensor_scalar(hmin, in0=hb, scalar1=0.0, scalar2=None, op0=Alu.min)
            hmax = smallp.tile([2 * C, BC], f32)
            nc.vector.tensor_scalar(hmax, in0=hb, scalar1=0.0, scalar2=None, op0=Alu.max)
            hsb = smallp.tile([2 * C, BC], f32)
            nc.vector.scalar_tensor_tensor(
                hsb, in0=hmin, scalar=0.1, in1=hmax, op0=Alu.mult, op1=Alu.add
            )
            ps_z = psmlpp.tile([BC, 1], f32, tag="mlp")
            nc.tensor.matmul(ps_z, lhsT=hsb, rhs=w2_sb, start=True, stop=True)
            zf = smallp.tile([BC, 1], f32)
            nc.scalar.activation(zf, ps_z, Act.Sigmoid, bias=b2_sb, scale=1.0)
            nc.sync.dma_start(out=out[:, :], in_=zf)

    nc.compile()
    return nc


def _get_nc():
    if "nc" not in _cache:
        _cache["nc"] = _build()
    return _cache["nc"]


def kernel(
    gh1, h1, gh2, h2, hpo_par, out_scale, out_bias,
    genes_hpo_scale, hpo_bias, gen_bias, W1, b1, W2, b2,
):
    from concourse.bass_utils import run_bass_kernel_spmd

    nc = _get_nc()
    vec = np.zeros((10, C), np.float32)
    vec[0] = out_scale
    vec[1] = out_bias
    vec[2] = genes_hpo_scale
    vec[3] = hpo_bias
    vec[4] = gen_bias
    vec[5:7] = np.asarray(b1, np.float32).reshape(2, C)
    vec[7, 0] = np.float32(np.asarray(b2).reshape(-1)[0])
    vec[8:10] = np.asarray(W2, np.float32).reshape(2, C)

    gh1 = np.ascontiguousarray(gh1, np.float32)
    h1 = np.ascontiguousarray(h1, np.float32)
    gh2 = np.ascontiguousarray(gh2, np.float32)
    h2 = np.ascontiguousarray(h2, np.float32)
    hpo = np.ascontiguousarray(hpo_par, np.float32)
    w1 = np.ascontiguousarray(W1, np.float32)

    in_maps = []
    for c in range(N_CORES):
        sl = slice(c * B_CORE, (c + 1) * B_CORE)
        in_maps.append(
            {
                "gh1": gh1[sl], "h1": h1[sl], "gh2": gh2[sl], "h2": h2[sl],
                "hpo": hpo, "w1": w1, "vec": vec,
            }
        )

    res = run_bass_kernel_spmd(nc, in_maps, core_ids=list(range(N_CORES)))
    outs = [res.results[c]["out"] for c in range(N_CORES)]
    return np.concatenate(outs, axis=0).astype(np.float32)
